# revision 2
# baseline (speedup 1.0000x reference)
"""Trainium2 Bass kernel for: y = x @ W.T; m = max(y, axis=1); out = broadcast(LSE(m) + log(B)).

Strategy (8 NeuronCores, data-parallel over batch):
  - Each core streams its x shard [65536, 512] f32 from HBM, casting to fp16
    during the DMA (SWDGE cast).
  - PE transposes each [128b, 128f] block (SBUF->PSUM, identity matmul), a
    DVE/ACT copy brings xT back to SBUF, then PE matmuls xT (stationary)
    against W.T chunks (moving) accumulating y [128b, 32] in PSUM fp32.
  - DVE row-max over the 32 outputs, ACT exp (+free-dim accumulate), DVE
    running sum -> per-core partial sums of exp(m) per partition [128, 1].
  - Host combines the 8x128 partials: l2 = log(sum) + log(B), broadcast.

The final output is a single scalar broadcast to [B, 1]; only the partial
sums are computed on device (the broadcast materialization is host-side).
"""

import math
from contextlib import ExitStack

import numpy as np

import concourse.bass as bass
import concourse.tile as tile
from concourse import bacc, mybir
from concourse import bass_utils
from concourse import masks

B = 524288
D = 512
O = 32
N_CORES = 8
B_LOC = B // N_CORES  # 65536
P = 128
KC = D // P  # 4 feature chunks


def build(b_loc: int = B_LOC, sup_rows: int = 1024, copy_split: int = 2):
    """Build + schedule the per-core program. Returns the compiled Bacc.

    sup_rows: rows per super-tile (one cast-DMA each), multiple of 128.
    copy_split: of every `copy_split` PSUM->SBUF copies, one goes to ACT
        (scalar), the rest to DVE (vector). 0 = all DVE.
    """
    blocks = sup_rows // P
    n_sup = b_loc // sup_rows
    assert n_sup * sup_rows == b_loc

    nc = bacc.Bacc(
        "TRN2", target_bir_lowering=False, debug=False, num_devices=N_CORES
    )
    xs = nc.dram_tensor("xs", [b_loc, D], mybir.dt.float32, kind="ExternalInput").ap()
    wt = nc.dram_tensor("wt", [KC, P, O], mybir.dt.float16, kind="ExternalInput").ap()
    acc_out = nc.dram_tensor(
        "acc_out", [P, 1], mybir.dt.float32, kind="ExternalOutput"
    ).ap()

    # [n_sup, 128, blocks*512]; partition p covers rows sup*sup_rows + p*blocks + i
    # (row order is irrelevant: we only need the sum of exp over all rows).
    xs_t = xs.rearrange("(s p i) f -> s p (i f)", p=P, i=blocks)

    with tile.TileContext(nc) as tc, ExitStack() as ctx:
        singles = ctx.enter_context(tc.tile_pool(name="singles", bufs=1))
        xpool = ctx.enter_context(tc.tile_pool(name="x16", bufs=3))
        tpool = ctx.enter_context(tc.tile_pool(name="xT", bufs=4))
        mpool = ctx.enter_context(tc.tile_pool(name="m8", bufs=3))
        ps_t = ctx.enter_context(
            tc.tile_pool(name="ps_t", bufs=3, space="PSUM")
        )
        ps_y = ctx.enter_context(
            tc.tile_pool(name="ps_y", bufs=3, space="PSUM")
        )

        wt_sb = singles.tile([P, KC, O], mybir.dt.float16)
        nc.sync.dma_start(out=wt_sb, in_=wt.rearrange("k p o -> p k o"))
        ident = singles.tile([P, P], mybir.dt.float16)
        masks.make_identity(nc, ident[:])
        acc = singles.tile([P, 1], mybir.dt.float32)
        nc.vector.memset(acc, 0.0)

        copy_i = 0
        for s in range(n_sup):
            x16 = xpool.tile([P, blocks, D], mybir.dt.float16)
            nc.gpsimd.dma_start(out=x16, in_=xs_t[s])  # fp32 -> fp16 cast DMA

            m8 = mpool.tile([P, blocks], mybir.dt.float32)
            for i in range(blocks):
                pst = ps_t.tile([P, KC, P], mybir.dt.float16)
                for k in range(KC):
                    nc.tensor.transpose(
                        pst[:, k, :], x16[:, i, k * P : (k + 1) * P], ident[:]
                    )
                xT = tpool.tile([P, KC, P], mybir.dt.float16)
                if copy_split and copy_i % copy_split == 0:
                    nc.scalar.copy(out=xT, in_=pst)
                else:
                    nc.vector.tensor_copy(out=xT, in_=pst)
                copy_i += 1

                psy = ps_y.tile([P, O], mybir.dt.float32)
                for k in range(KC):
                    nc.tensor.matmul(
                        psy,
                        lhsT=xT[:, k, :],
                        rhs=wt_sb[:, k, :],
                        start=(k == 0),
                        stop=(k == KC - 1),
                    )
                nc.vector.tensor_reduce(
                    out=m8[:, i : i + 1],
                    in_=psy,
                    axis=mybir.AxisListType.X,
                    op=mybir.AluOpType.max,
                )

            e8 = mpool.tile([P, blocks], mybir.dt.float32)
            esum = mpool.tile([P, 1], mybir.dt.float32)
            nc.scalar.activation(
                out=e8,
                in_=m8,
                func=mybir.ActivationFunctionType.Exp,
                accum_out=esum,
            )
            nc.vector.tensor_add(acc, acc, esum)

        nc.sync.dma_start(out=acc_out, in_=acc)

    nc.compile()
    return nc


_CACHE: dict = {}


def _get_nc(**kw):
    key = tuple(sorted(kw.items()))
    if key not in _CACHE:
        _CACHE[key] = build(**kw)
    return _CACHE[key]


def _host_prep_w(W: np.ndarray) -> np.ndarray:
    # W [32, 512] f32 -> W.T chunks [4, 128, 32] fp16
    return np.ascontiguousarray(W.T.reshape(KC, P, O)).astype(np.float16)


def kernel(x: np.ndarray, W: np.ndarray) -> np.ndarray:
    assert x.shape == (B, D) and W.shape == (O, D)
    nc = _get_nc()
    wt = _host_prep_w(W)
    x = np.ascontiguousarray(x, dtype=np.float32)
    in_maps = [
        {"xs": x[c * B_LOC : (c + 1) * B_LOC], "wt": wt} for c in range(N_CORES)
    ]
    res = bass_utils.run_bass_kernel_spmd(nc, in_maps, core_ids=list(range(N_CORES)))
    total = np.float64(0.0)
    for r in res.results:
        total += r["acc_out"].astype(np.float64).sum()
    l2 = math.log(total) + math.log(B)
    return np.full((B, 1), np.float32(l2), dtype=np.float32)
